# revision 10
# baseline (speedup 1.0000x reference)
"""Causal self-attention Trainium2 kernel (8 NeuronCores, SPMD).

Sharding: 8 cores = 4 batches x 2 head-groups. Each core computes, for its
(batch b, head-group g): Q/K/V projections restricted to g's 8 heads
(column-parallel), causal attention for those heads, and the partial output
projection ctx_g @ Wo[g rows] (row-parallel). Host sums the two partials per
batch and adds the bias terms (bv @ Wo + bo).

All matmuls run in bf16 with fp32 PSUM accumulation. Attention uses the
transposed-scores orientation: scoresT[k, q] tiles are exp'd in place and fed
directly as the moving operand of the PV matmul (no PE transposes, all
matmuls N=512). Softmax skips the max subtraction (scores are ~N(0,1); exp
cannot overflow) and folds the 1/sqrt(dh) scale into the exp activation.

v2 scheduling: matmuls are grouped so consecutive instructions share one
stationary operand (4 moving tiles per weight load in the projections, the
ones-vector loaded once per normalizer group), PSUM evictions are batched
into single wide ACT/DVE instructions spanning 2-4 banks, and the softmax
reciprocal runs 128-partition-parallel on a broadcast copy of the normalizer
instead of serially on one partition.
"""

import sys

sys.path.insert(0, "/opt/trn_rl_repo")

from contextlib import ExitStack

import numpy as np

import concourse.bass as bass
import concourse.tile as tile
from concourse import bass_isa, mybir
from concourse.bass_utils import run_bass_kernel_spmd

BF16 = mybir.dt.bfloat16
F32 = mybir.dt.float32
NP_BF16 = mybir.dt.np(BF16)

# Problem constants (hardcoded per contract).
B = 4          # batch
S = 2048       # sequence length
DM = 2048      # d_model
H = 16         # total heads
HD = 128       # head dim
G = 2          # head groups (tensor parallel degree)
NHL = H // G   # local heads per core
DHL = NHL * HD # local head dims
NCORES = 8
P = 128        # partitions
FD = 512       # matmul moving free dim (one PSUM bank of f32)
SCALE = 1.0 / float(np.sqrt(HD))
MASK_VAL = -1e30

# Opcodes whose walrus lowering handles multi-wait sync itself (or that we
# must not touch). Everything else gets its waits normalized to <= 1.
_WAIT_EXEMPT = {
    "NoOp",
    "EventSemaphore",
    "UnconditionalBranch",
    "RegisterMove",
    "ISA",
    "TileRelease",
}


def _fix_sync_waits(nc, max_waits=1):
    """Hoist extra sync-waits onto single-wait NoOps on the issuing engine.

    Several walrus instruction encodings (PSEUDO_DMA_DIRECT2D, S3_LW, CTRL_NO,
    ...) have a single sync-wait slot and fail codegen with "Too many sync
    wait commands" when Tile attaches more. A NoOp on the same engine
    immediately before the instruction performs the extra wait at the
    sequencer, which is semantically identical.
    """
    f = nc.m.functions[0]
    fixed = 0

    def walk(blocks):
        nonlocal fixed
        for b in blocks:
            il = b.instructions
            i = 0
            while i < len(il):
                inst = il[i]
                si = getattr(inst, "sync_info", None)
                ow = list(si.on_wait) if si is not None and si.on_wait else []
                if inst.opcode not in _WAIT_EXEMPT and len(ow) > max_waits:
                    keep = ow[len(ow) - max_waits :]
                    extra = ow[: len(ow) - max_waits]
                    for j, w in enumerate(extra):
                        nop = mybir.InstNoOp(
                            name=f"{inst.name}_waitfix{j}",
                            engine=inst.engine,
                            ins=[],
                            outs=[],
                            bass_nofuse=True,
                            sync_info=mybir.SyncInfo(on_wait=[w], on_update=[]),
                        )
                        il.insert(i, nop)
                        i += 1
                    inst.sync_info = mybir.SyncInfo(
                        on_wait=keep,
                        on_update=list(si.on_update) if si.on_update else [],
                    )
                    fixed += 1
                i += 1
            walk(getattr(b, "blocks", []) or [])

    walk(f.blocks)
    return fixed


def _bcast_ap(ap, nparts):
    """Partition-broadcast view of a single-partition AP."""
    return bass.AP(
        tensor=ap.tensor, offset=ap.offset, ap=[[0, nparts]] + list(ap.ap[1:])
    )


def build_nc(seq=S, dm=DM, nhl=NHL, fix_waits=True):
    """Build the single-core Bass program (same program for all 8 cores)."""
    dhl = nhl * P
    nkc = dm // P    # contraction chunks for projections
    nst = seq // P   # seq tiles
    nqb = seq // FD  # 512-wide q blocks
    nsc = seq // FD  # 512-wide seq chunks

    nc = bass.Bass()
    # All inputs are pre-arranged on the host into SBUF-friendly layouts so
    # every DMA is contiguous per partition line.
    xT_d = nc.dram_tensor("xT", [P, nkc, seq], BF16, kind="ExternalInput")
    wq_d = nc.dram_tensor("wq", [nhl, P, nkc, P], BF16, kind="ExternalInput")
    wk_d = nc.dram_tensor("wk", [nhl, P, nkc, P], BF16, kind="ExternalInput")
    wv_d = nc.dram_tensor("wv", [P, nkc, dhl], BF16, kind="ExternalInput")
    wo_d = nc.dram_tensor("wo", [P, dhl // P, dm], BF16, kind="ExternalInput")
    bqk_d = nc.dram_tensor("bqk", [P, 2, nhl], F32, kind="ExternalInput")
    out_d = nc.dram_tensor("out", [seq, dm], F32, kind="ExternalOutput")

    with tile.TileContext(nc) as tc:
        es_qkv = ExitStack()
        qkv = es_qkv.enter_context(tc.tile_pool(name="qkv", bufs=1))
        QT = qkv.tile([P, nhl, seq], BF16)   # [hd-within-head, h, seq]
        KT = qkv.tile([P, nhl, seq], BF16)
        V = qkv.tile([P, nst, dhl], BF16)    # [seq-within-tile, st, dv]

        consts = es_qkv.enter_context(tc.tile_pool(name="consts", bufs=1))
        bqk_sb = consts.tile([P, 2, nhl], F32)
        ones_sb = consts.tile([P, 1], BF16)
        nc.vector.memset(ones_sb[:, :], 1.0)
        # Upper-keep mask for the transposed diagonal block:
        # umask[k, q] = 0 if q >= k else MASK_VAL.
        umask = consts.tile([P, P], F32)
        nc.gpsimd.memset(umask[:, :], 0.0)
        nc.gpsimd.affine_select(
            out=umask[:, :],
            in_=umask[:, :],
            compare_op=mybir.AluOpType.is_ge,
            fill=MASK_VAL,
            base=0,
            pattern=[[1, P]],
            channel_multiplier=-1,
        )
        nc.gpsimd.dma_start(out=bqk_sb[:, :, :], in_=bqk_d[:, :, :])

        # Strip pool opened early so the first strips prefetch during the x
        # load. Lives on the right-side stack, which is empty until phase 3.
        es_strip = ExitStack()
        spool = es_strip.enter_context(
            tc.tile_pool(name="spool", bufs=3, side="right")
        )
        strips = {}

        def load_strip(which, h):
            wd = wq_d if which == 0 else wk_d
            ws = spool.tile([P, nkc, P], BF16, tag="ws", name=f"ws{which}_{h}")
            nc.gpsimd.dma_start(out=ws[:, :, :], in_=wd[h, :, :, :])
            strips[(which, h)] = ws

        es_x = ExitStack()
        xpool = es_x.enter_context(tc.tile_pool(name="xpool", bufs=1))
        xT = xpool.tile([P, nkc, seq], BF16)
        for i in range(nkc):
            nc.sync.dma_start(out=xT[:, i : i + 1, :], in_=xT_d[:, i : i + 1, :])

        load_strip(0, 0)
        load_strip(0, 1)

        es_ppsum = ExitStack()
        ppsum = es_ppsum.enter_context(
            tc.tile_pool(name="ppsum", bufs=2, space="PSUM")
        )
        # ---------------- Phase 2: QT = (x@Wq)^T then KT = (x@Wk)^T ---------
        # Two passes (Q then K). Per head: one 4-bank PSUM quad, stationary
        # strip chunk loaded once per contraction step and reused for all 4
        # seq chunks; one 2048-wide ACT eviction per head.
        for which, DST in ((0, QT), (1, KT)):
            for h in range(nhl):
                ws = strips.pop((which, h))
                nxt = (which, h + 2) if h + 2 < nhl else (which + 1, h + 2 - nhl)
                if nxt[0] <= 1:
                    load_strip(*nxt)
                quad = ppsum.tile([P, nsc, FD], F32, tag="pquad", bufs=2)
                for c in range(nkc):
                    for sc in range(nsc):
                        nc.tensor.matmul(
                            quad[:, sc, :],
                            ws[:, c, :],
                            xT[:, c, sc * FD : (sc + 1) * FD],
                            start=(c == 0),
                            stop=(c == nkc - 1),
                        )
                nc.scalar.activation(
                    DST[:, h, :],
                    quad[:, :, :],
                    mybir.ActivationFunctionType.Identity,
                    bias=bqk_sb[:, which, h : h + 1],
                )
        es_strip.close()
        es_wv = ExitStack()
        wvpool = es_wv.enter_context(tc.tile_pool(name="wvpool", bufs=1))
        wv_sb = wvpool.tile([P, nkc, dhl], BF16)
        wstep = min(2, nkc)
        for i in range(0, nkc, wstep):
            nc.gpsimd.dma_start(
                out=wv_sb[:, i : i + wstep, :], in_=wv_d[:, i : i + wstep, :]
            )

        # ---------------- Phase 1b: V = x @ Wv  ([seq, dhl] layout) ---------
        # Two seq tiles per 4-bank PSUM quad (same tag as P2 so PSUM stays
        # within 8 banks); stationary x tile loaded once per contraction step
        # and reused for both dv chunks; one 2048-wide DVE eviction per pair.
        ndc = dhl // FD
        for stp in range(0, nst, 2):
            vv = ppsum.tile([P, nsc, FD], F32, tag="pquad", bufs=2)
            for c in range(nkc):
                for so in range(2):
                    st = stp + so
                    for dc in range(ndc):
                        nc.tensor.matmul(
                            vv[:, so * ndc + dc, :],
                            xT[:, c, st * P : (st + 1) * P],
                            wv_sb[:, c, dc * FD : (dc + 1) * FD],
                            start=(c == 0),
                            stop=(c == nkc - 1),
                        )
            nc.vector.tensor_copy(V[:, stp : stp + 2, :], vv[:, :, :])
        es_wv.close()
        es_ppsum.close()
        es_x.close()

        # ---------------- Phase 3: causal attention (transposed scores) ------
        # Per (h, qb): scoresT[k, qb*512:(qb+1)*512] computed per k-tile into
        # 2-bank PSUM pairs, exp'd with one 1024-wide ACT per pair, consumed
        # directly as the moving operand of the PV matmul. Normalizer: ones
        # matmuls (single weight load) accumulate csum, DMA-bounce broadcast,
        # then a 128-partition-parallel reciprocal and multiply on DVE.
        es_ctxT = ExitStack()
        ctxTpool = es_ctxT.enter_context(
            tc.tile_pool(name="ctxTpool", bufs=1, side="right")
        )
        # One ctx tile per q block so phase 4 tiles only depend on the pv
        # steps that actually wrote their q range.
        ctxT = [
            ctxTpool.tile([P, nhl, FD], BF16, name=f"ctxT{qb}") for qb in range(nqb)
        ]
        # Prefetch wo during attention (right side, persists into P4).
        es_proj = ExitStack()
        wopool = es_proj.enter_context(
            tc.tile_pool(name="wopool", bufs=1, side="right")
        )
        wo_sb = wopool.tile([P, dhl // P, dm], BF16)
        ostep = min(2, dhl // P)
        for i in range(0, dhl // P, ostep):
            nc.gpsimd.dma_start(
                out=wo_sb[:, i : i + ostep, :], in_=wo_d[:, i : i + ostep, :]
            )

        es_attn = ExitStack()
        spsum = es_attn.enter_context(tc.tile_pool(name="spsum", bufs=2, space="PSUM"))
        cpsum = es_attn.enter_context(tc.tile_pool(name="cpsum", bufs=2, space="PSUM"))
        apool = es_attn.enter_context(tc.tile_pool(name="apool", bufs=2))
        npool = es_attn.enter_context(tc.tile_pool(name="npool", bufs=2))
        npsum = es_attn.enter_context(tc.tile_pool(name="npsum", bufs=2, space="PSUM"))
        dpool = es_attn.enter_context(tc.tile_pool(name="dpool", bufs=3, space="DRAM"))

        state = {}

        def stage_scores(h, qb):
            kmax = (qb + 1) * (FD // P)  # k-tiles for this q block
            exp_sb = apool.tile([P, nst, FD], BF16, tag="exp", name=f"exp{h}_{qb}")
            memsets = []
            for t in range(kmax // 2):
                ps = spsum.tile(
                    [P, 2, FD], F32, tag="spair", bufs=2, name=f"sps{t}"
                )
                for s in range(2):
                    kt = 2 * t + s
                    nc.tensor.matmul(
                        ps[:, s, :],
                        KT[:, h, kt * P : (kt + 1) * P],
                        QT[:, h, qb * FD : (qb + 1) * FD],
                        start=True,
                        stop=True,
                    )
                    j = kt - 4 * qb
                    if j >= 0:
                        # diagonal block: keep q >= k within the block
                        nc.vector.tensor_add(
                            ps[:, s, j * P : (j + 1) * P],
                            ps[:, s, j * P : (j + 1) * P],
                            umask[:, :],
                        )
                nc.scalar.activation(
                    exp_sb[:, 2 * t : 2 * t + 2, :],
                    ps[:, :, :],
                    mybir.ActivationFunctionType.Exp,
                    scale=SCALE,
                )
                for s in range(2):
                    kt = 2 * t + s
                    j = kt - 4 * qb
                    if j > 0:
                        # q < k region of partial diagonal tiles: weight 0
                        memsets.append((kt, j))
            for kt, j in memsets:
                nc.vector.memset(exp_sb[:, kt, : j * P], 0.0)
            state[(h, qb)] = (exp_sb, kmax)

        def stage_pv(h, qb, last=False):
            exp_sb, kmax = state.pop((h, qb))
            pv = cpsum.tile([P, FD], F32, tag="pv", bufs=2, name=f"pv{h}_{qb}")
            csum = npsum.tile([1, FD], F32, tag="csum", bufs=2, name=f"cs{h}_{qb}")
            for kt in range(kmax):
                nc.tensor.matmul(
                    pv[:, :],
                    V[:, kt, h * P : (h + 1) * P],
                    exp_sb[:, kt, :],
                    start=(kt == 0),
                    stop=(kt == kmax - 1),
                )
            # csum matmuls grouped so the ones stationary loads once.
            for kt in range(kmax):
                nc.tensor.matmul(
                    csum[:, :],
                    ones_sb[:, :],
                    exp_sb[:, kt, :],
                    start=(kt == 0),
                    stop=(kt == kmax - 1),
                )
            cs_sb = npool.tile([1, FD], F32, tag="cs_sb", name=f"css{h}_{qb}")
            # Evict csum PSUM->SBUF (GpSimd cannot read PSUM). The reciprocal
            # is free-size-bound on DVE, so bounce the 512 sums through DRAM
            # into a [128, 4] layout, invert there, then bounce back out to a
            # [128, 512] partition-broadcast for the multiply.
            nc.scalar.copy(cs_sb[:, :], csum[:, :])
            pv_src = pv
            if last:
                # Final step: evict pv to SBUF so all PSUM banks release
                # before phase 4 starts; the slow normalizer chain then
                # overlaps phase 4's first tiles.
                pv_sb = npool.tile([P, FD], F32, tag="pv_sb")
                nc.scalar.copy(pv_sb[:, :], pv[:, :])
                pv_src = pv_sb
            rd = dpool.tile([1, FD], F32, tag="rd", name=f"rd{h}_{qb}")
            nc.sync.dma_start(out=rd[:, :], in_=cs_sb[:, :])
            r4 = npool.tile([P, FD // P], F32, tag="r4", name=f"r4{h}_{qb}")
            rr4 = npool.tile([P, FD // P], F32, tag="rr4", name=f"rr4{h}_{qb}")
            r4_view = bass.AP(
                tensor=rd.tensor, offset=rd.offset, ap=[[FD // P, P], [1, FD // P]]
            )
            nc.sync.dma_start(out=r4[:, :], in_=r4_view)
            nc.vector.reciprocal(rr4[:, :], r4[:, :])
            rd2 = dpool.tile([1, FD], F32, tag="rd2", name=f"rd2{h}_{qb}")
            rd2_view = bass.AP(
                tensor=rd2.tensor, offset=rd2.offset, ap=[[FD // P, P], [1, FD // P]]
            )
            nc.sync.dma_start(out=rd2_view, in_=rr4[:, :])
            rbc = npool.tile([P, FD], F32, tag="rbc", name=f"rbc{h}_{qb}")
            nc.sync.dma_start(out=rbc[:, :], in_=_bcast_ap(rd2[:, :], P))
            nc.vector.tensor_mul(ctxT[qb][:, h, :], pv_src[:, :], rbc[:, :])

        # qb order (1,2,3,0): each step's exp eviction time then matches the
        # tensor-engine work of the preceding pv step, so ScalarE never gates.
        steps = [(h, qb) for h in range(nhl) for qb in (1, 2, 3, 0)]
        for i, (h, qb) in enumerate(steps):
            stage_scores(h, qb)
            if i > 0:
                stage_pv(*steps[i - 1])
        stage_pv(*steps[-1], last=True)
        es_attn.close()
        es_qkv.close()

        # ---------------- Phase 4: out = ctx @ Wo ---------------------------
        # Stationary ctx tile loaded once per contraction step, reused for
        # all 4 output chunks; one 2048-wide ACT eviction + one store per
        # seq tile.
        nmc = dm // FD
        opsum = es_proj.enter_context(tc.tile_pool(name="opsum", bufs=2, space="PSUM"))
        opool = es_proj.enter_context(
            tc.tile_pool(name="opool", bufs=2, side="right")
        )
        # q blocks in pv completion order (1, 2, 3, 0) so the first tiles
        # never wait on the final pv step's normalizer chain.
        for qb in (1, 2, 3, 0):
            for si in range(FD // P):
                st = qb * (FD // P) + si
                op = opsum.tile([P, nmc, FD], F32, tag="oquad", bufs=2)
                for dc in range(dhl // P):
                    for mc in range(nmc):
                        nc.tensor.matmul(
                            op[:, mc, :],
                            ctxT[qb][:, dc, si * P : (si + 1) * P],
                            wo_sb[:, dc, mc * FD : (mc + 1) * FD],
                            start=(dc == 0),
                            stop=(dc == dhl // P - 1),
                        )
                ot = opool.tile([P, dm], F32, tag="ot")
                nc.scalar.copy(ot[:, :], op[:, :, :])
                nc.sync.dma_start(
                    out=out_d[st * P : (st + 1) * P, :],
                    in_=ot[:, :],
                )
        es_proj.close()
        es_ctxT.close()

    if fix_waits:
        _fix_sync_waits(nc)
    return nc


def shard_inputs(x, Wq, bq, Wk, bk, Wv, bv, Wo, bo, seq=S, dm=DM, nhl=NHL, nb=B, g_=G):
    """Host-side sharding: returns per-core input maps (bf16 pre-arranged)."""
    dhl = nhl * P
    nkc = dm // P
    xTs = []
    for b in range(nb):
        xt = np.ascontiguousarray(x[b].T).astype(NP_BF16)  # [dm, seq]
        xTs.append(np.ascontiguousarray(xt.reshape(nkc, P, seq).transpose(1, 0, 2)))
    wqs, wks, wvs, wos, bqks = [], [], [], [], []
    for g in range(g_):
        sl = slice(g * dhl, (g + 1) * dhl)
        wq_s = Wq[:, sl].astype(NP_BF16)
        wk_s = Wk[:, sl].astype(NP_BF16)
        wv_s = Wv[:, sl].astype(NP_BF16)
        wo_s = Wo[sl, :].astype(NP_BF16)
        # wq/wk: [nhl, P, nkc, P] strip-major
        wqs.append(
            np.ascontiguousarray(wq_s.reshape(nkc, P, nhl, P).transpose(2, 1, 0, 3))
        )
        wks.append(
            np.ascontiguousarray(wk_s.reshape(nkc, P, nhl, P).transpose(2, 1, 0, 3))
        )
        wvs.append(np.ascontiguousarray(wv_s.reshape(nkc, P, dhl).transpose(1, 0, 2)))
        wos.append(
            np.ascontiguousarray(wo_s.reshape(dhl // P, P, dm).transpose(1, 0, 2))
        )
        bqk = np.stack(
            [
                np.asarray(bq[sl], np.float32).reshape(nhl, P),
                np.asarray(bk[sl], np.float32).reshape(nhl, P),
            ]
        )  # [2, nhl, P]
        bqks.append(np.ascontiguousarray(bqk.transpose(2, 0, 1)))  # [P, 2, nhl]
    in_maps = []
    for c in range(nb * g_):
        b, g = divmod(c, g_)
        in_maps.append(
            {
                "xT": xTs[b],
                "wq": wqs[g],
                "wk": wks[g],
                "wv": wvs[g],
                "wo": wos[g],
                "bqk": bqks[g],
            }
        )
    return in_maps


_CACHE = {}


def _get_nc():
    if "nc" not in _CACHE:
        _CACHE["nc"] = build_nc()
    return _CACHE["nc"]


def run(inputs, trace=False):
    """Run the SPMD kernel; returns (full_output, BassKernelResults)."""
    inputs = {k: np.asarray(v) for k, v in inputs.items()}
    nc = _get_nc()
    in_maps = shard_inputs(**inputs)
    res = run_bass_kernel_spmd(
        nc, in_maps, core_ids=list(range(NCORES)), trace=trace
    )
    Wo = np.asarray(inputs["Wo"], np.float32)
    const_row = (
        np.asarray(inputs["bv"], np.float32) @ Wo + np.asarray(inputs["bo"], np.float32)
    )
    out = np.empty((B, S, DM), np.float32)
    for b in range(B):
        out[b] = res.results[G * b]["out"] + res.results[G * b + 1]["out"] + const_row
    return out, res


def kernel(**inputs):
    out, _ = run(inputs, trace=False)
    return out
